# revision 1
# baseline (speedup 1.0000x reference)
# kernel.py — Bidirectional masked-GRU-with-predictor on 8 Trainium2 NeuronCores.
#
# Problem (reference.py): B=128, T=1024, H=512
#   per step, per direction:
#     x_in = where(mask, predictor(h), x)            predictor: Linear(H,H)->ReLU->Linear(H,1)->Tanh
#     h    = GRUCell(h, x_in)                        PyTorch gate order (r, z, n)
#   output [B, T, 2H] = concat(fwd hidden states, time-reversed bwd hidden states)
#
# Sharding: 8 cores = 2 directions x 4 batch groups of 32.  All cores run the
# SAME Bass program; per-core data differs (bwd cores get time-reversed x/mask
# and their outputs are flipped back on the host).
#
# On-core layout ("feature-major, chunk-in-free"):
#   h^T kept as [128 partitions = feature%128, (j,b)] where j = feature//128 (4 chunks),
#   b = local batch (32).  Big matmul: stationary = W^T 128x128 blocks (fp16, FWL),
#   moving = h chunks; gates + predictor-hidden land in PSUM feature-major, so the
#   new h is produced directly in the layout the next step's matmul consumes.
#   fp16 matmul inputs + fp32 PSUM accumulate + fp32 vector math:
#   measured emulation error vs fp32 reference: ~7e-4 of output absmax.

import numpy as np

B, T, H = 128, 1024, 512
NCORES = 8
BL = B // 4          # 32: batch per core (4 groups x 2 directions)
KC = H // 128        # 4 contraction chunks
MC = (3 * H + H) // 128  # 16 output chunks (w_hh 12 + p_w1 4)
U_DEF = 64           # time steps per For_i iteration

_cache = {}


def _build_program(t_steps=T, u_steps=U_DEF, bl=BL, n_cores=NCORES, repeat=1):
    import concourse.bacc as bacc
    import concourse.bass as bass
    import concourse.tile as tile
    from concourse.tile import add_dep_helper
    from concourse import mybir

    f16 = mybir.dt.float16
    f32 = mybir.dt.float32

    nc = bacc.Bacc(
        "TRN2",
        target_bir_lowering=False,
        debug=False,
        enable_asserts=False,
        num_devices=n_cores,
    )

    # ---- DRAM tensors (per-core data; same names on every core) ----
    d_wt = nc.dram_tensor("wt", [128, MC * KC * 128], f16, kind="ExternalInput").ap()
    d_gi = nc.dram_tensor("gilhs", [2, 12 * 128], f16, kind="ExternalInput").ap()
    d_bc = nc.dram_tensor("bcols", [4, 5 * 128], f16, kind="ExternalInput").ap()
    d_e4 = nc.dram_tensor("e4", [4, KC * bl], f16, kind="ExternalInput").ap()
    d_pw2 = nc.dram_tensor("pw2t", [128, KC], f16, kind="ExternalInput").ap()
    d_pb2 = nc.dram_tensor("pb2", [1, 1], f32, kind="ExternalInput").ap()
    d_a = nc.dram_tensor("a_arr", [t_steps, bl], f16, kind="ExternalInput").ap()
    d_m = nc.dram_tensor("m_arr", [t_steps, bl], f16, kind="ExternalInput").ap()
    out_t = t_steps if repeat == 1 else u_steps
    d_out = nc.dram_tensor(
        "outl", [out_t, 128, KC, bl], f16, kind="ExternalOutput"
    ).ap()

    Relu = mybir.ActivationFunctionType.Relu
    Tanh = mybir.ActivationFunctionType.Tanh
    Sigmoid = mybir.ActivationFunctionType.Sigmoid

    with tile.TileContext(nc) as tc:
        import contextlib

        with contextlib.ExitStack() as ctx:
            consts = ctx.enter_context(tc.tile_pool(name="consts", bufs=1))
            psum = ctx.enter_context(tc.tile_pool(name="psum", bufs=1, space="PSUM"))
            work = ctx.enter_context(tc.tile_pool(name="work", bufs=2))
            io = ctx.enter_context(tc.tile_pool(name="io", bufs=2))

            # ---- constant preload ----
            WT = consts.tile([128, MC * KC * 128], f16, tag="WT")
            GIL = consts.tile([2, 12 * 128], f16, tag="GIL")
            BC = consts.tile([4, 5 * 128], f16, tag="BC")
            E4 = consts.tile([4, KC * bl], f16, tag="E4")
            PW2 = consts.tile([128, KC], f16, tag="PW2")
            PB2 = consts.tile([1, 1], f32, tag="PB2")
            for dst, src in (
                (WT, d_wt), (GIL, d_gi), (BC, d_bc),
                (E4, d_e4), (PW2, d_pw2), (PB2, d_pb2),
            ):
                nc.sync.dma_start(out=dst, in_=src)

            # persistent ping-pong hidden state, fp16, [128, (j,b)]
            h0 = consts.tile([128, KC * bl], f16, tag="h0")
            h1 = consts.tile([128, KC * bl], f16, tag="h1")
            nc.vector.memset(h0, 0.0)
            nc.vector.memset(h1, 0.0)
            h_tiles = [h0, h1]

            # persistent PSUM accumulators (single-buffered; readers drain early)
            G_r = psum.tile([128, KC * bl], f32, tag="G_r")
            G_z = psum.tile([128, KC * bl], f32, tag="G_z")
            G_n = psum.tile([128, KC * bl], f32, tag="G_n")
            PHp = psum.tile([128, KC * bl], f32, tag="PH")
            GIN = psum.tile([128, KC * bl], f32, tag="GIN")
            PREN = psum.tile([128, KC * bl], f32, tag="PREN")
            PRD = psum.tile([1, bl], f32, tag="PRD")

            def w_block(m, k):
                bi = m * KC + k
                return WT[:, bi * 128:(bi + 1) * 128]

            def emit_region(g_idx, region, h_cur, has_gi=False):
                # bias matmul opens the accumulation (start=True covers the
                # whole region), then 4 m-chunks x 4 k-chunks of W blocks.
                # Returns (first, last) instruction for PE-order chaining.
                first = nc.tensor.matmul(
                    region, BC[:, g_idx * 128:(g_idx + 1) * 128], E4,
                    start=True, stop=False, skip_group_check=True,
                )
                base_m = g_idx * KC if g_idx < 3 else 12
                last = first
                for j in range(KC):
                    m = base_m + j
                    for k in range(KC):
                        last = nc.tensor.matmul(
                            region[:, j * bl:(j + 1) * bl],
                            w_block(m, k),
                            h_cur[:, k * bl:(k + 1) * bl],
                            start=False,
                            stop=(not has_gi and k == KC - 1),
                            skip_group_check=True,
                        )
                return first, last

            def emit_gi(g_idx, region, gi_rhs):
                # K=2 rank-1 matmuls: region[:, j] += w_ih_g[j] (x) x_in
                first = last = None
                for j in range(KC):
                    gj = g_idx * KC + j
                    last = nc.tensor.matmul(
                        region[:, j * bl:(j + 1) * bl],
                        GIL[:, gj * 128:(gj + 1) * 128],
                        gi_rhs,
                        start=False, stop=True, skip_group_check=True,
                    )
                    if first is None:
                        first = last
                return first, last

            def pe_order(a_first, b_last):
                # force PE issue order: a runs after b (ordering only)
                add_dep_helper(a_first.ins, b_last.ins, sync=False)

            def step(u, h_cur, h_new, S2, MB, t_dyn):
                gi_rhs = S2[:, u * bl:(u + 1) * bl]
                # PE order: PH, W_r, PRD, W_n, gi_r, GIN, W_z, gi_z
                ph_f, ph_l = emit_region(3, PHp, h_cur)
                r_f, r_l = emit_region(0, G_r, h_cur, has_gi=True)
                pe_order(r_f, ph_l)
                relu = work.tile([128, KC * bl], f16, tag="relu")
                nc.vector.tensor_scalar_max(relu, PHp, 0.0)
                prd_f = prd_l = None
                for k in range(KC):
                    prd_l = nc.tensor.matmul(
                        PRD, PW2[:, k:k + 1], relu[:, k * bl:(k + 1) * bl],
                        start=(k == 0), stop=(k == KC - 1), skip_group_check=True,
                    )
                    if prd_f is None:
                        prd_f = prd_l
                pe_order(prd_f, r_l)
                pred = work.tile([1, bl], f16, tag="pred")
                nc.scalar.activation(out=pred, in_=PRD, func=Tanh, bias=PB2[:, :])
                nc.vector.tensor_mul(
                    S2[0:1, u * bl:(u + 1) * bl], pred, MB[0:1, u * bl:(u + 1) * bl]
                )

                n_f, n_l = emit_region(2, G_n, h_cur)
                pe_order(n_f, prd_l)
                gir_f, gir_l = emit_gi(0, G_r, gi_rhs)
                pe_order(gir_f, n_l)
                gin_f = nc.tensor.matmul(
                    GIN, BC[:, 4 * 128:5 * 128], E4,
                    start=True, stop=False, skip_group_check=True)
                pe_order(gin_f, gir_l)
                _, gin_l = emit_gi(2, GIN, gi_rhs)
                z_f, z_l = emit_region(1, G_z, h_cur, has_gi=True)
                pe_order(z_f, gin_l)
                giz_f, _ = emit_gi(1, G_z, gi_rhs)
                pe_order(giz_f, z_l)

                r_sb = work.tile([128, KC * bl], f16, tag="r_sb")
                nc.scalar.activation(out=r_sb, in_=G_r, func=Sigmoid)

                # n = tanh(GIN + r * G_n)
                u_n = work.tile([128, KC * bl], f32, tag="u_n")
                nc.vector.tensor_mul(u_n, r_sb, G_n)
                nc.vector.tensor_add(PREN, u_n, GIN)
                n_sb = work.tile([128, KC * bl], f16, tag="n_sb")
                nc.scalar.activation(out=n_sb, in_=PREN, func=Tanh)

                z_sb = work.tile([128, KC * bl], f16, tag="z_sb")
                nc.scalar.activation(out=z_sb, in_=G_z, func=Sigmoid)

                # h' = z*h - (z-1)*n  == z*h + (1-z)*n
                t1 = work.tile([128, KC * bl], f16, tag="t1")
                nc.vector.tensor_mul(t1, z_sb, h_cur)
                t2 = work.tile([128, KC * bl], f16, tag="t2")
                nc.vector.scalar_tensor_tensor(
                    out=t2, in0=z_sb, scalar=1.0, in1=n_sb,
                    op0=mybir.AluOpType.subtract, op1=mybir.AluOpType.mult,
                )
                nc.vector.tensor_sub(h_new, t1, t2)

                # stream h' out:  outl[t, p, j, b]
                dst = d_out[bass.ds(t_dyn, 1)].rearrange("o p j b -> (o p) j b")
                nc.sync.dma_start(
                    out=dst, in_=h_new.rearrange("p (j b) -> p j b", b=bl)
                )

            n_blocks = t_steps // u_steps
            with tc.For_i(
                0, n_blocks * repeat, 1, hint_engines=(mybir.EngineType.PE,)
            ) as iv:
                S2 = io.tile([2, u_steps * bl], f16, tag="S2")
                MB = io.tile([1, u_steps * bl], f16, tag="MB")
                toff = (iv * u_steps) if repeat == 1 else 0
                nc.sync.dma_start(
                    out=S2[1:2, :].rearrange("p (u b) -> p u b", b=bl),
                    in_=d_a[bass.ds(toff, u_steps)].unsqueeze(0),
                )
                nc.sync.dma_start(
                    out=MB[0:1, :].rearrange("p (u b) -> p u b", b=bl),
                    in_=d_m[bass.ds(toff, u_steps)].unsqueeze(0),
                )
                for u in range(u_steps):
                    step(
                        u,
                        h_tiles[u % 2],
                        h_tiles[(u + 1) % 2],
                        S2,
                        MB,
                        (iv * u_steps + u) if repeat == 1 else u,
                    )

    nc.compile()
    return nc


def _prep_core_inputs(inputs, core, t_steps=T, bl=BL):
    """Build the per-core input map (numpy) for core id `core`."""
    f16 = np.float16
    direction = 0 if core < 4 else 1  # 0 fwd, 1 bwd
    bg = core % 4
    sl = slice(bg * bl, (bg + 1) * bl)

    x = np.asarray(inputs["x"], np.float32)[:, :, 0]      # [B, T]
    msk = np.asarray(inputs["mask"]).astype(np.float32)[:, :, 0]
    pfx = "wf" if direction == 0 else "wb"
    w_ih = np.asarray(inputs[f"{pfx}_ih"], np.float32)[:, 0]   # [3H]
    w_hh = np.asarray(inputs[f"{pfx}_hh"], np.float32)         # [3H, H]
    b_ih = np.asarray(inputs[f"b{pfx[1]}_ih"], np.float32)
    b_hh = np.asarray(inputs[f"b{pfx[1]}_hh"], np.float32)
    p_w1 = np.asarray(inputs["p_w1"], np.float32)
    p_b1 = np.asarray(inputs["p_b1"], np.float32)
    p_w2 = np.asarray(inputs["p_w2"], np.float32)
    p_b2 = np.asarray(inputs["p_b2"], np.float32)

    xs = x[sl].T.copy()      # [T, bl]
    ms = msk[sl].T.copy()
    if direction == 1:
        xs = xs[::-1].copy()
        ms = ms[::-1].copy()
    a_arr = (xs * (1.0 - ms)).astype(f16)
    m_arr = ms.astype(f16)

    W = np.concatenate([w_hh, p_w1], axis=0)             # [2048, 512]
    Wr = W.reshape(MC, 128, KC, 128)                     # [m, c, k, p]
    wt = Wr.transpose(3, 0, 2, 1).reshape(128, MC * KC * 128).astype(f16)

    # gi stationaries: per (gate g, chunk j) a [2,128] block, both rows =
    # w_ih[g*512 + j*128 : ...]; contract with [tmp; a] rows of S2.
    gilhs = np.broadcast_to(
        w_ih.reshape(3 * KC, 128)[None, :, :], (2, 3 * KC, 128)
    ).reshape(2, 12 * 128).astype(f16).copy()

    bias_regions = [
        b_ih[0:H] + b_hh[0:H],          # r
        b_ih[H:2 * H] + b_hh[H:2 * H],  # z
        b_hh[2 * H:3 * H],              # n: b_hh only
        p_b1,                           # ph
        b_ih[2 * H:3 * H],              # gin: b_ih_n
    ]
    bcols = np.concatenate(
        [br.reshape(KC, 128) for br in bias_regions], axis=1
    ).astype(f16)                                        # [4, 5*128]

    e4 = np.zeros((KC, KC, bl), np.float32)
    for j in range(KC):
        e4[j, j, :] = 1.0
    e4 = e4.reshape(KC, KC * bl).astype(f16)

    pw2t = p_w2[0].reshape(KC, 128).T.astype(f16).copy()
    pb2 = p_b2.reshape(1, 1).astype(np.float32)

    return {
        "wt": wt, "gilhs": gilhs, "bcols": bcols, "e4": e4,
        "pw2t": pw2t, "pb2": pb2,
        "a_arr": a_arr[:t_steps], "m_arr": m_arr[:t_steps],
    }


def _assemble(results, t_steps=T, bl=BL):
    """results: list of 8 per-core dicts with 'outl' [T,128,KC,bl] fp16."""
    out = np.zeros((B, t_steps, 2 * H), np.float32)
    for core in range(NCORES):
        direction = 0 if core < 4 else 1
        bg = core % 4
        arr = np.asarray(results[core]["outl"], np.float16).astype(np.float32)
        # [t, p, j, b] -> [b, t, j, p] -> [b, t, 512]
        arr = arr.transpose(3, 0, 2, 1).reshape(bl, t_steps, H)
        if direction == 1:
            arr = arr[:, ::-1]
        out[bg * bl:(bg + 1) * bl, :, direction * H:(direction + 1) * H] = arr
    return out


def kernel(**inputs):
    from concourse.bass_utils import run_bass_kernel_spmd

    key = (T, U_DEF, BL)
    if key not in _cache:
        _cache[key] = _build_program(T, U_DEF, BL)
    nc = _cache[key]

    in_maps = [_prep_core_inputs(inputs, c) for c in range(NCORES)]
    res = run_bass_kernel_spmd(
        nc, in_maps, core_ids=list(range(NCORES)), trace=False
    )
    return _assemble(res.results)



# revision 8
# speedup vs baseline: 1.0047x; 1.0047x over previous
# kernel.py — Bidirectional masked-GRU-with-predictor on 8 Trainium2 NeuronCores.
#
# Problem (reference.py): B=128, T=1024, H=512
#   per step, per direction:
#     x_in = where(mask, predictor(h), x)            predictor: Linear(H,H)->ReLU->Linear(H,1)->Tanh
#     h    = GRUCell(h, x_in)                        PyTorch gate order (r, z, n)
#   output [B, T, 2H] = concat(fwd hidden states, time-reversed bwd hidden states)
#
# Sharding: 8 cores = 2 directions x 4 batch groups of 32.  All cores run the
# SAME Bass program; per-core data differs (bwd cores get time-reversed x/mask
# and their outputs are flipped back on the host).
#
# On-core layout ("feature-major, chunk-in-free"):
#   h^T kept as [128 partitions = feature%128, (j,b)] where j = feature//128 (4 chunks),
#   b = local batch (32).  Big matmul: stationary = W^T 128x128 blocks (fp16, FWL),
#   moving = h chunks; gates + predictor-hidden land in PSUM feature-major, so the
#   new h is produced directly in the layout the next step's matmul consumes.
#
# Latency optimizations over the v1 kernel:
#   - PSUM regions double-buffered by step parity; bank layout chosen so no
#     PSUM bank is read (Act/DVE) while the PE is still writing other columns
#     of the same bank
#   - r/z/GIN biases folded into the gi rank-1 matmuls via a 3-row stationary
#     {w*pred_m, w*a, bias*ones} (removes 3 group-opening bias matmuls)
#   - h' written low-half first; next step's k={0,1} matmuls depend only on
#     that range, so they start while the high half is still finishing

import numpy as np

B, T, H = 128, 1024, 512
NCORES = 8
BL = B // 4          # 32: batch per core (4 groups x 2 directions)
KC = H // 128        # 4 contraction chunks
MC = (3 * H + H) // 128  # 16 output chunks (w_hh 12 + p_w1 4)
U_DEF = 64           # time steps per For_i iteration

_cache = {}


def _build_program(t_steps=T, u_steps=U_DEF, bl=BL, n_cores=NCORES, repeat=1):
    import concourse.bacc as bacc
    import concourse.bass as bass
    import concourse.tile as tile
    from concourse.tile import add_dep_helper
    from concourse import mybir

    f16 = mybir.dt.float16
    f32 = mybir.dt.float32

    nc = bacc.Bacc(
        "TRN2",
        target_bir_lowering=False,
        debug=False,
        enable_asserts=False,
        num_devices=n_cores,
    )

    # ---- DRAM tensors (per-core data; same names on every core) ----
    d_wt = nc.dram_tensor("wt", [128, MC * KC * 128], f16, kind="ExternalInput").ap()
    d_gi3 = nc.dram_tensor("gil3", [3, 12 * 128], f16, kind="ExternalInput").ap()
    d_bc = nc.dram_tensor("bcols", [4, 2 * 128], f16, kind="ExternalInput").ap()
    d_e4 = nc.dram_tensor("e4", [4, KC * bl], f16, kind="ExternalInput").ap()
    d_pw2 = nc.dram_tensor("pw2t", [128, KC], f16, kind="ExternalInput").ap()
    d_pb2 = nc.dram_tensor("pb2", [1, 1], f32, kind="ExternalInput").ap()
    d_ones = nc.dram_tensor("ones_row", [1, U_DEF * bl], f16, kind="ExternalInput").ap()
    d_a = nc.dram_tensor("a_arr", [t_steps, bl], f16, kind="ExternalInput").ap()
    d_m = nc.dram_tensor("m_arr", [t_steps, bl], f16, kind="ExternalInput").ap()
    out_t = t_steps if repeat == 1 else u_steps
    d_out = nc.dram_tensor(
        "outl", [out_t, 128, KC, bl], f16, kind="ExternalOutput"
    ).ap()

    Tanh = mybir.ActivationFunctionType.Tanh
    Sigmoid = mybir.ActivationFunctionType.Sigmoid
    sub_ = mybir.AluOpType.subtract
    mul_ = mybir.AluOpType.mult

    HB = 2 * bl          # 64: half of the (j,b) free dim
    FB = KC * bl         # 128: full (j,b) free dim

    with tile.TileContext(nc) as tc:
        import contextlib

        with contextlib.ExitStack() as ctx:
            consts = ctx.enter_context(tc.tile_pool(name="consts", bufs=1))
            psum = ctx.enter_context(tc.tile_pool(name="psum", bufs=1, space="PSUM"))
            work = ctx.enter_context(tc.tile_pool(name="work", bufs=2))
            io = ctx.enter_context(tc.tile_pool(name="io", bufs=2))

            # ---- constant preload ----
            WT = consts.tile([128, MC * KC * 128], f16, tag="WT")
            GIL3 = consts.tile([3, 12 * 128], f16, tag="GIL3")
            BC = consts.tile([4, 2 * 128], f16, tag="BC")
            E4 = consts.tile([4, FB], f16, tag="E4")
            PW2 = consts.tile([128, KC], f16, tag="PW2")
            PB2 = consts.tile([1, 1], f32, tag="PB2")
            for dst, src in (
                (WT, d_wt), (GIL3, d_gi3), (BC, d_bc),
                (E4, d_e4), (PW2, d_pw2), (PB2, d_pb2),
            ):
                nc.sync.dma_start(out=dst, in_=src)

            # persistent ping-pong hidden state, fp16, [128, (j,b)]
            h0 = consts.tile([128, FB], f16, tag="h0")
            h1 = consts.tile([128, FB], f16, tag="h1")
            nc.vector.memset(h0, 0.0)
            nc.vector.memset(h1, 0.0)
            h_tiles = [h0, h1]

            # PSUM, double-buffered by step parity, 3 banks per parity:
            #   PRZ: [G_r | G_z]      PNB: [G_n | GIN | PRD]   PHC: [PH | PREN]
            # Layout rule: no bank is read while the PE still writes other
            # columns of it.
            PRZ0 = psum.tile([128, 256], f32, tag="PRZ0")
            PRZ1 = psum.tile([128, 256], f32, tag="PRZ1")
            PNB0 = psum.tile([128, 256], f32, tag="PNB0")
            PNB1 = psum.tile([128, 256], f32, tag="PNB1")
            PHC0 = psum.tile([128, 256], f32, tag="PHC0")
            PHC1 = psum.tile([128, 256], f32, tag="PHC1")
            PRD0 = psum.tile([1, 32], f32, tag="PRD0")
            PRD1 = psum.tile([1, 32], f32, tag="PRD1")
            PRZ_t = [PRZ0, PRZ1]
            PNB_t = [PNB0, PNB1]
            PHC_t = [PHC0, PHC1]
            PRD_t = [PRD0, PRD1]

            def w_block(m, k):
                bi = m * KC + k
                return WT[:, bi * 128:(bi + 1) * 128]

            def gi_block(g, j):
                bi = g * KC + j
                return GIL3[:, bi * 128:(bi + 1) * 128]

            def step(u, h_cur, h_new, S3, MB, t_dyn):
                par = u % 2
                PRZ = PRZ_t[par]
                PNB = PNB_t[par]
                PHC = PHC_t[par]
                GN = PNB[:, 0:128]
                GIN = PNB[:, 128:256]
                PRD = PRD_t[par]
                PH = PHC[:, 0:128]
                PREN = PHC[:, 128:256]

                s3_mv = S3[:, u * bl:(u + 1) * bl]

                pe = []  # emitted PE instructions, chained in order

                def mm(out, lhsT, rhs, start, stop):
                    i = nc.tensor.matmul(
                        out, lhsT, rhs, start=start, stop=stop,
                        skip_group_check=True,
                    )
                    pe.append(i)
                    return i

                def wmm(psum_ap, col0, m_base, m, k, start, stop):
                    j = m - m_base
                    mm(
                        psum_ap[:, col0 + j * bl: col0 + (j + 1) * bl],
                        w_block(m, k),
                        h_cur[:, k * bl:(k + 1) * bl],
                        start, stop,
                    )

                # ---------- PE stream (strictly in-order engine!) ----------
                # PH: bias then 16 blocks, k01 before k23 (h' halves land split)
                mm(PH, BC[:, 128:256], E4, True, False)
                for m in range(12, 16):
                    for k in (0, 1):
                        wmm(PHC, 0, 12, m, k, False, False)
                for m in range(12, 16):
                    for k in (2, 3):
                        wmm(PHC, 0, 12, m, k, False, False)
                # relu(PH) full width [DVE]
                relu = work.tile([128, FB], f16, tag="relu")
                nc.vector.tensor_scalar_max(relu, PH, 0.0)

                # W_r
                for m in range(0, 4):
                    for k in (0, 1):
                        wmm(PRZ, 0, 0, m, k, k == 0, False)
                for m in range(0, 4):
                    for k in (2, 3):
                        wmm(PRZ, 0, 0, m, k, False, False)
                # PRD (4 consecutive; relu is ready by the time PE gets here)
                for k in range(KC):
                    mm(PRD, PW2[:, k:k + 1], relu[:, k * bl:(k + 1) * bl],
                       k == 0, k == KC - 1)
                # pred = tanh(PRD + b2)  [Act]
                pred = work.tile([1, bl], f16, tag="pred")
                nc.scalar.activation(out=pred, in_=PRD, func=Tanh, bias=PB2[:, :])
                # S3 row0 = pred * mask  [DVE]
                nc.vector.tensor_mul(
                    S3[0:1, u * bl:(u + 1) * bl], pred, MB[0:1, u * bl:(u + 1) * bl]
                )

                # W_z
                for m in range(4, 8):
                    for k in (0, 1):
                        wmm(PRZ, 128, 4, m, k, k == 0, False)
                for m in range(4, 8):
                    for k in (2, 3):
                        wmm(PRZ, 128, 4, m, k, False, False)
                # G_n: bias then 16 blocks
                mm(GN, BC[:, 0:128], E4, True, False)
                for m in range(8, 12):
                    for k in (0, 1):
                        wmm(PNB, 0, 8, m, k, False, False)
                for m in range(8, 12):
                    for k in (2, 3):
                        wmm(PNB, 0, 8, m, k, False, m == 11 and k == 3)

                # gi rank-1s: rows {pred*m, a, ones} x {w, w, bias}
                # gi_z first so sig_r never reads PRZ while gi_z writes it
                for j in range(KC):  # g=1 (z), closes G_z
                    mm(PRZ[:, 128 + j * bl:128 + (j + 1) * bl], gi_block(1, j),
                       s3_mv, False, True)
                for j in range(KC):  # g=0 (r), closes G_r (sig_r starts ASAP)
                    mm(PRZ[:, j * bl:(j + 1) * bl], gi_block(0, j), s3_mv,
                       False, True)
                for j in range(KC):  # g=2 (n) -> GIN (sole writer)
                    mm(PNB[:, 128 + j * bl:128 + (j + 1) * bl], gi_block(2, j),
                       s3_mv, True, True)

                # chain PE issue order
                for a, b in zip(pe[1:], pe[:-1]):
                    add_dep_helper(a.ins, b.ins, sync=False)

                # ---------- elementwise tail ----------
                rz = work.tile([128, 256], f16, tag="rz")
                r_sb = rz[:, 0:128]
                z_sb = rz[:, 128:256]
                nc.scalar.activation(out=r_sb, in_=PRZ[:, 0:128], func=Sigmoid)
                nc.scalar.activation(out=z_sb, in_=PRZ[:, 128:256], func=Sigmoid)

                # n = tanh(GIN + r*G_n)
                u_n = work.tile([128, FB], f32, tag="u_n")
                nc.vector.tensor_mul(u_n, r_sb, GN)
                nc.vector.tensor_add(PREN, u_n, GIN)
                n_sb = work.tile([128, FB], f16, tag="n_sb")
                nc.scalar.activation(out=n_sb, in_=PREN, func=Tanh)

                # t1 = z * h [DVE, slots into the hole while tanh_n runs]
                t1 = work.tile([128, FB], f16, tag="t1")
                nc.vector.tensor_mul(t1, z_sb, h_cur)

                # h' = t1 - (z-1)*n, low half first
                t2 = work.tile([128, FB], f16, tag="t2")
                nc.vector.scalar_tensor_tensor(
                    out=t2[:, 0:HB], in0=z_sb[:, 0:HB], scalar=1.0,
                    in1=n_sb[:, 0:HB], op0=sub_, op1=mul_,
                )
                nc.vector.tensor_sub(h_new[:, 0:HB], t1[:, 0:HB], t2[:, 0:HB])
                nc.vector.scalar_tensor_tensor(
                    out=t2[:, HB:FB], in0=z_sb[:, HB:FB], scalar=1.0,
                    in1=n_sb[:, HB:FB], op0=sub_, op1=mul_,
                )
                nc.vector.tensor_sub(h_new[:, HB:FB], t1[:, HB:FB], t2[:, HB:FB])

                # stream h' out:  outl[t, p, j, b]
                dst = d_out[bass.ds(t_dyn, 1)].rearrange("o p j b -> (o p) j b")
                nc.sync.dma_start(
                    out=dst, in_=h_new.rearrange("p (j b) -> p j b", b=bl)
                )

            n_blocks = t_steps // u_steps
            with tc.For_i(
                0, n_blocks * repeat, 1, hint_engines=(mybir.EngineType.PE,)
            ) as iv:
                S3 = io.tile([3, u_steps * bl], f16, tag="S3")
                MB = io.tile([1, u_steps * bl], f16, tag="MB")
                toff = (iv * u_steps) if repeat == 1 else 0
                nc.sync.dma_start(
                    out=S3[1:2, :].rearrange("p (u b) -> p u b", b=bl),
                    in_=d_a[bass.ds(toff, u_steps)].unsqueeze(0),
                )
                nc.sync.dma_start(out=S3[2:3, :], in_=d_ones)
                nc.sync.dma_start(
                    out=MB[0:1, :].rearrange("p (u b) -> p u b", b=bl),
                    in_=d_m[bass.ds(toff, u_steps)].unsqueeze(0),
                )
                for u in range(u_steps):
                    step(
                        u,
                        h_tiles[u % 2],
                        h_tiles[(u + 1) % 2],
                        S3,
                        MB,
                        (iv * u_steps + u) if repeat == 1 else u,
                    )

    nc.compile()
    return nc


def _prep_core_inputs(inputs, core, t_steps=T, bl=BL):
    """Build the per-core input map (numpy) for core id `core`."""
    f16 = np.float16
    direction = 0 if core < 4 else 1  # 0 fwd, 1 bwd
    bg = core % 4
    sl = slice(bg * bl, (bg + 1) * bl)

    x = np.asarray(inputs["x"], np.float32)[:, :, 0]      # [B, T]
    msk = np.asarray(inputs["mask"]).astype(np.float32)[:, :, 0]
    pfx = "wf" if direction == 0 else "wb"
    w_ih = np.asarray(inputs[f"{pfx}_ih"], np.float32)[:, 0]   # [3H]
    w_hh = np.asarray(inputs[f"{pfx}_hh"], np.float32)         # [3H, H]
    b_ih = np.asarray(inputs[f"b{pfx[1]}_ih"], np.float32)
    b_hh = np.asarray(inputs[f"b{pfx[1]}_hh"], np.float32)
    p_w1 = np.asarray(inputs["p_w1"], np.float32)
    p_b1 = np.asarray(inputs["p_b1"], np.float32)
    p_w2 = np.asarray(inputs["p_w2"], np.float32)
    p_b2 = np.asarray(inputs["p_b2"], np.float32)

    xs = x[sl].T.copy()      # [T, bl]
    ms = msk[sl].T.copy()
    if direction == 1:
        xs = xs[::-1].copy()
        ms = ms[::-1].copy()
    a_arr = (xs * (1.0 - ms)).astype(f16)
    m_arr = ms.astype(f16)

    W = np.concatenate([w_hh, p_w1], axis=0)             # [2048, 512]
    Wr = W.reshape(MC, 128, KC, 128)                     # [m, c, k, p]
    wt = Wr.transpose(3, 0, 2, 1).reshape(128, MC * KC * 128).astype(f16)

    # gi stationaries: per (gate g, chunk j) a [3,128] block:
    #   row0 = w_ih chunk (x pred*m), row1 = w_ih chunk (x a), row2 = bias (x 1)
    bias_rows = np.concatenate([
        (b_ih[0:H] + b_hh[0:H]),          # r
        (b_ih[H:2 * H] + b_hh[H:2 * H]),  # z
        b_ih[2 * H:3 * H],                # n (GIN part)
    ])                                    # [1536]
    wchunks = w_ih.reshape(12, 128)
    bchunks = bias_rows.reshape(12, 128)
    gil3 = np.stack([wchunks, wchunks, bchunks], axis=1)   # [12, 3, 128]
    gil3 = gil3.transpose(1, 0, 2).reshape(3, 12 * 128).astype(f16).copy()

    bias_regions = [
        b_hh[2 * H:3 * H],              # G_n: b_hh only
        p_b1,                           # PH
    ]
    bcols = np.concatenate(
        [br.reshape(KC, 128) for br in bias_regions], axis=1
    ).astype(f16)                                        # [4, 2*128]

    e4 = np.zeros((KC, KC, bl), np.float32)
    for j in range(KC):
        e4[j, j, :] = 1.0
    e4 = e4.reshape(KC, KC * bl).astype(f16)

    pw2t = p_w2[0].reshape(KC, 128).T.astype(f16).copy()
    pb2 = p_b2.reshape(1, 1).astype(np.float32)
    ones_row = np.ones((1, U_DEF * bl), f16)

    return {
        "wt": wt, "gil3": gil3, "bcols": bcols, "e4": e4,
        "pw2t": pw2t, "pb2": pb2, "ones_row": ones_row,
        "a_arr": a_arr[:t_steps], "m_arr": m_arr[:t_steps],
    }


def _assemble(results, t_steps=T, bl=BL):
    """results: list of 8 per-core dicts with 'outl' [T,128,KC,bl] fp16."""
    out = np.zeros((B, t_steps, 2 * H), np.float32)
    for core in range(NCORES):
        direction = 0 if core < 4 else 1
        bg = core % 4
        arr = np.asarray(results[core]["outl"], np.float16).astype(np.float32)
        # [t, p, j, b] -> [b, t, j, p] -> [b, t, 512]
        arr = arr.transpose(3, 0, 2, 1).reshape(bl, t_steps, H)
        if direction == 1:
            arr = arr[:, ::-1]
        out[bg * bl:(bg + 1) * bl, :, direction * H:(direction + 1) * H] = arr
    return out


def kernel(**inputs):
    from concourse.bass_utils import run_bass_kernel_spmd

    key = (T, U_DEF, BL)
    if key not in _cache:
        _cache[key] = _build_program(T, U_DEF, BL)
    nc = _cache[key]

    in_maps = [_prep_core_inputs(inputs, c) for c in range(NCORES)]
    res = run_bass_kernel_spmd(
        nc, in_maps, core_ids=list(range(NCORES)), trace=False
    )
    return _assemble(res.results)
